# revision 1
# baseline (speedup 1.0000x reference)
"""Block-sparse linear y = x @ W^T + b on 8 TRN2 NeuronCores.

Problem shape (hardcoded): x [8192, 4096] f32, weight [1024, 64, 64] f32
(64x64 blocks), bias [4096] f32, row_idx/col_idx [1024] int32 over a 64x64
block grid.

Strategy: data-parallel over tokens (1024/core). Each core computes
y^T[feat, tok] = W x^T + b via K=64/M=64 block matmuls packed 4-wide into
the PE-array quadrants with tile_position; compute in bf16 (fp32 PSUM
accumulate), x^T resident in SBUF (two partition-phase copies so either
array row-group can serve any col-block), weights streamed as grouped DMA
transfers, bias added on evacuation via ScalarE, output stored as y^T f32
and transposed/concatenated on host.

TRN2 PSUM rule (measured): concurrent matmuls that share an output
col-group must write different PSUM banks -> row-group ki writes psum
tile[ki]; evacuation computes A + B + bias.
"""

from contextlib import ExitStack
from dataclasses import dataclass, field

import numpy as np
import ml_dtypes

import concourse.tile as tile
from concourse import bacc, mybir
from concourse.bass_utils import run_bass_kernel_spmd

BLK = 64
OUT_BLK = 64
IN_BLK = 64
D_IN = IN_BLK * BLK    # 4096
D_OUT = OUT_BLK * BLK  # 4096
N_CORES = 8
WGRP = 16              # weight tiles per DMA group
XCH = 2                # x tiles (128-row groups) per DMA chunk
BF16 = ml_dtypes.bfloat16


@dataclass
class _WTile:
    ki0: tuple | None = None   # (mi, c, w_idx) served by array rows 0-63
    ki1: tuple | None = None   # (mi, c, w_idx) served by array rows 64-127


@dataclass
class _Pair:
    r: tuple
    wtiles: list = field(default_factory=list)
    adjacent: bool = False


def _make_schedule(row_idx, col_idx):
    # keep-last dedupe of (r, c), matching jax .at[].set semantics
    d = {}
    for i in range(len(row_idx)):
        d[(int(row_idx[i]), int(col_idx[i]))] = i
    blocks_by_r = [[] for _ in range(OUT_BLK)]
    for (r, c), w in d.items():
        blocks_by_r[r].append((c, w))
    for lst in blocks_by_r:
        lst.sort()
    pairs = []
    for p in range(OUT_BLK // 2):
        r1, r2 = 2 * p, 2 * p + 1
        ps = _Pair(r=(r1, r2), adjacent=True)
        q = [
            [(0, c, w) for (c, w) in blocks_by_r[r1]],
            [(1, c, w) for (c, w) in blocks_by_r[r2]],
        ]
        t = 0
        while q[0] or q[1]:
            first = t % 2
            a = q[first].pop() if q[first] else (
                q[1 - first].pop() if q[1 - first] else None)
            b = q[1 - first].pop() if q[1 - first] else (
                q[first].pop() if q[first] else None)
            ps.wtiles.append(_WTile(ki0=a, ki1=b))
            t += 1
        pairs.append(ps)
    n_wtiles = sum(len(ps.wtiles) for ps in pairs)
    return pairs, n_wtiles


def _pack_host_arrays(weight, bias, pairs):
    n_wtiles = sum(len(ps.wtiles) for ps in pairs)
    n_groups = (n_wtiles + WGRP - 1) // WGRP
    wgrp = np.zeros((max(n_groups, 1), 128, WGRP * BLK), dtype=BF16)
    bias_pk = np.zeros((128, len(pairs)), dtype=np.float32)
    wT = np.ascontiguousarray(
        np.transpose(np.asarray(weight), (0, 2, 1))).astype(BF16)
    t = 0
    for p, ps in enumerate(pairs):
        r1, r2 = ps.r
        bias_pk[:64, p] = bias[r1 * BLK:(r1 + 1) * BLK]
        bias_pk[64:, p] = bias[r2 * BLK:(r2 + 1) * BLK]
        for wt in ps.wtiles:
            g, j = divmod(t, WGRP)
            for ki, half in ((0, wt.ki0), (1, wt.ki1)):
                if half is not None:
                    wgrp[g, ki * 64:(ki + 1) * 64,
                         j * BLK:(j + 1) * BLK] = wT[half[2]]
            t += 1
    return wgrp, bias_pk


def _x_tile_of(c, ki):
    """(copy, tile index) of the resident x^T tile serving block c on array
    row-group ki. Copy 'a' holds blocks (2b, 2b+1) on partition halves
    (0, 1); copy 'b' is shifted 64 rows: (2b+1, 2b+2), with tile 31
    wrapping to block 0."""
    if (c % 2) == ki:
        return ("a", c // 2)
    if c % 2 == 1:
        return ("b", (c - 1) // 2)
    return ("b", (c // 2 - 1) % (IN_BLK // 2))


def _build_kernel(pairs, n_wtiles, ntok, w_bufs=10, ps_bufs=8, out_bufs=6):
    assert ntok % 512 == 0
    n_th = ntok // 512
    sdt = mybir.dt.bfloat16
    f32 = mybir.dt.float32

    nc = bacc.Bacc("TRN2", target_bir_lowering=False, debug=False)
    xt_rows = D_IN + 64
    n_groups = (n_wtiles + WGRP - 1) // WGRP
    xt_d = nc.dram_tensor("xt", [xt_rows, ntok], sdt,
                          kind="ExternalInput").ap()
    wg_d = nc.dram_tensor("wgrp", [max(n_groups, 1), 128, WGRP * BLK], sdt,
                          kind="ExternalInput").ap()
    bias_d = nc.dram_tensor("bias_pk", [128, len(pairs)], f32,
                            kind="ExternalInput").ap()
    yt_d = nc.dram_tensor("yt", [D_OUT, ntok], f32,
                          kind="ExternalOutput").ap()

    with tile.TileContext(nc) as tc:
        with ExitStack() as ctx:
            xpool = ctx.enter_context(tc.tile_pool(name="xp", bufs=1))
            wpool = ctx.enter_context(tc.tile_pool(name="wp", bufs=w_bufs))
            pspool = ctx.enter_context(
                tc.tile_pool(name="ps", bufs=ps_bufs, space="PSUM"))
            opool = ctx.enter_context(tc.tile_pool(name="op", bufs=out_bufs))
            bpool = ctx.enter_context(tc.tile_pool(name="bp", bufs=1))

            bias_sb = bpool.tile([128, len(pairs)], f32, tag="bias",
                                 name="bias_sb")
            nc.sync.dma_start(bias_sb[:], bias_d[:])

            xchunks = {}

            def x_ap(c, ki, th):
                cp, b = _x_tile_of(c, ki)
                cb, wi = divmod(b, XCH)
                key = (cp, cb)
                if key not in xchunks:
                    t = xpool.tile([128, XCH * ntok], sdt, tag=f"x{cp}{cb}",
                                   name=f"x{cp}{cb}")
                    off = cb * 128 * XCH + (64 if cp == "b" else 0)
                    src = xt_d[off:off + 128 * XCH, :].rearrange(
                        "(c p) t -> p c t", p=128)
                    dst = t[:].rearrange("p (c t) -> p c t", c=XCH)
                    nc.sync.dma_start(dst, src)
                    xchunks[key] = t
                t = xchunks[key]
                o = wi * ntok + th * 512
                return t[ki * 64:(ki + 1) * 64, o:o + 512]

            nmm = [{(ki, mi): 0 for ki in (0, 1) for mi in (0, 1)}
                   for _ in pairs]
            for p, ps_ in enumerate(pairs):
                for wt in ps_.wtiles:
                    for ki, half in ((0, wt.ki0), (1, wt.ki1)):
                        if half is not None:
                            nmm[p][(ki, half[0])] += 1
            done = [{(th, ki, mi): 0 for th in range(n_th)
                     for ki in (0, 1) for mi in (0, 1)} for _ in pairs]

            psum = {}
            wg_tiles = {}

            def ensure_psum(p, th):
                if (p, th) not in psum:
                    psum[(p, th)] = [
                        pspool.tile([128, 512], f32, tag="ps",
                                    name=f"ps{p}_{th}_{k}") for k in range(2)]

            def store_out(p, th, osb):
                ps_ = pairs[p]
                ts = slice(th * 512, (th + 1) * 512)
                r1, r2 = ps_.r
                if ps_.adjacent:
                    nc.gpsimd.dma_start(yt_d[r1 * BLK:r1 * BLK + 128, ts],
                                        osb[:])
                else:
                    nc.gpsimd.dma_start(yt_d[r1 * BLK:(r1 + 1) * BLK, ts],
                                        osb[0:64, :])
                    nc.gpsimd.dma_start(yt_d[r2 * BLK:(r2 + 1) * BLK, ts],
                                        osb[64:128, :])

            def eviction_th(p, th):
                osb = opool.tile([128, 512], f32, tag="o32",
                                 name=f"o{p}_{th}")
                pt = psum.pop((p, th))
                if all(v > 0 for v in nmm[p].values()):
                    nc.scalar.activation(
                        osb[:], pt[0][:],
                        mybir.ActivationFunctionType.Identity,
                        bias=bias_sb[:, p:p + 1], scale=1.0)
                    nc.vector.tensor_add(osb[:], osb[:], pt[1][:])
                else:
                    for mi in (0, 1):
                        oh = osb[mi * 64:(mi + 1) * 64, :]
                        bh = bias_sb[mi * 64:(mi + 1) * 64, p:p + 1]
                        srcs = [pt[ki][mi * 64:(mi + 1) * 64, :]
                                for ki in (0, 1) if nmm[p][(ki, mi)] > 0]
                        if not srcs:
                            nc.vector.memset(oh, 0.0)
                            nc.vector.tensor_scalar_add(oh, oh, bh)
                        else:
                            nc.scalar.activation(
                                oh, srcs[0],
                                mybir.ActivationFunctionType.Identity,
                                bias=bh, scale=1.0)
                            if len(srcs) > 1:
                                nc.vector.tensor_add(oh, oh, srcs[1])
                store_out(p, th, osb)

            pair_base = []
            acc = 0
            for ps_ in pairs:
                pair_base.append(acc)
                acc += len(ps_.wtiles)

            for p, ps_ in enumerate(pairs):
                if not ps_.wtiles:
                    continue
                for th in range(n_th):
                    ensure_psum(p, th)
                for wt_j, wt in enumerate(ps_.wtiles):
                    idx = pair_base[p] + wt_j
                    gi, jj = divmod(idx, WGRP)
                    for gpf in (gi, gi + 1, gi + 2):
                        if gpf < n_groups and gpf not in wg_tiles:
                            wg_tiles[gpf] = wpool.tile(
                                [128, WGRP * BLK], sdt, tag="wg",
                                name=f"wg{gpf}")
                            nc.sync.dma_start(wg_tiles[gpf][:],
                                              wg_d[gpf, :, :])
                    for ki, half in ((0, wt.ki0), (1, wt.ki1)):
                        if half is None:
                            continue
                        mi, c, w = half
                        lhsT = wg_tiles[gi][ki * 64:(ki + 1) * 64,
                                            jj * BLK:(jj + 1) * BLK]
                        for th in range(n_th):
                            done[p][(th, ki, mi)] += 1
                            first = done[p][(th, ki, mi)] == 1
                            last = done[p][(th, ki, mi)] == nmm[p][(ki, mi)]
                            nc.tensor.matmul(
                                psum[(p, th)][ki][mi * 64:(mi + 1) * 64, :],
                                lhsT, x_ap(c, ki, th),
                                start=first, stop=last,
                                tile_position=(ki * 64, mi * 64),
                                skip_group_check=True,
                            )
                for th in range(n_th):
                    eviction_th(p, th)

            for p, ps_ in enumerate(pairs):
                if ps_.wtiles:
                    continue
                for th in range(n_th):
                    osb = opool.tile([128, 512], f32, tag="o32",
                                     name=f"oz{p}_{th}")
                    nc.vector.memset(osb[:], 0.0)
                    nc.vector.tensor_scalar_add(osb[:], osb[:],
                                                bias_sb[:, p:p + 1])
                    store_out(p, th, osb)
    nc.compile()
    return nc


def kernel(x, weight, bias, row_idx, col_idx):
    x = np.asarray(x, dtype=np.float32)
    weight = np.asarray(weight, dtype=np.float32)
    bias = np.asarray(bias, dtype=np.float32)
    row_idx = np.asarray(row_idx)
    col_idx = np.asarray(col_idx)
    ntok_total = x.shape[0]
    assert ntok_total % N_CORES == 0
    ntok = ntok_total // N_CORES

    pairs, n_wt = _make_schedule(row_idx, col_idx)
    wgrp, bias_pk = _pack_host_arrays(weight, bias, pairs)
    nc = _build_kernel(pairs, n_wt, ntok)

    in_maps = []
    for c in range(N_CORES):
        xt = np.ascontiguousarray(
            x[c * ntok:(c + 1) * ntok].T).astype(BF16)
        xt = np.concatenate([xt, xt[:64]], axis=0)
        in_maps.append({"xt": xt, "wgrp": wgrp, "bias_pk": bias_pk})

    res = run_bass_kernel_spmd(nc, in_maps, core_ids=list(range(N_CORES)))
    y = np.empty((ntok_total, D_OUT), dtype=np.float32)
    for c in range(N_CORES):
        y[c * ntok:(c + 1) * ntok] = res.results[c]["yt"].T
    return y



# revision 7
# speedup vs baseline: 1.0011x; 1.0011x over previous
"""Block-sparse linear y = x @ W^T + b on 8 TRN2 NeuronCores.

Problem shape (hardcoded): x [8192, 4096] f32, weight [1024, 64, 64] f32
(64x64 blocks), bias [4096] f32, row_idx/col_idx [1024] int32 over a 64x64
block grid.

Strategy: data-parallel over tokens (1024/core). Each core computes
y^T[feat, tok] = W x^T + b with 64x64 block matmuls packed 4-wide into the
PE-array quadrants via tile_position. Measured HW behavior: each quadrant
sustains one 512-row bf16 matmul per ~216ns with four quadrants fully
concurrent, so the schedule's job is to keep all four quadrant queues
non-empty, especially during the input-DMA phase (~390 GB/s shared).

Design (vs. the 178us version):
  - single x^T copy in SBUF (block column c lives on partition half c%2);
    the PE-array row-group (ki) of each block is forced by column parity,
    with a few blocks reassigned via duplicated columns to balance the
    global even/odd load. x tiles are DMA'd in pair-first-use order.
  - all weights stay SBUF-resident (8.5MB), one DMA per [128, 2048] group
    in first-use order.
  - fp16 output, stored per row-pair in pair-permuted DRAM layout (one
    [128, ntok] store per pair), un-permuted on host.
  - build-time greedy scheduler with an arrival-time model emits matmuls
    in x-arrival order across 4 open (pair, token-half) PSUM units.
  - DMA issue instructions (~650ns each) spread across engines: x on
    sync+scalar+vector, weights+stores on gpsimd.
"""

from contextlib import ExitStack

import numpy as np
import ml_dtypes

import concourse.tile as tile
from concourse import bacc, mybir
from concourse.bass_utils import run_bass_kernel_spmd

BLK = 64
OUT_BLK = 64
IN_BLK = 64
D_IN = IN_BLK * BLK    # 4096
D_OUT = OUT_BLK * BLK  # 4096
N_CORES = 8
BF16 = ml_dtypes.bfloat16

WSLOTS = 32            # weight slots per group per partition half
MM_NS = 216.0          # steady per-quadrant matmul cadence
EVICT_NS = 1500.0      # modeled eviction latency (psum free)
XBW = 260.0            # bytes/ns across the two x queues (~260 GB/s)
WBW = 130.0            # bytes/ns for the weight queue (~130 GB/s)
T_BOOT = 2500.0        # modeled queue-bootstrap offset


# ----------------------------------------------------------------- planning

def _dedupe(row_idx, col_idx):
    d = {}
    for i in range(len(row_idx)):
        d[(int(row_idx[i]), int(col_idx[i]))] = i
    blocks_by_r = [[] for _ in range(OUT_BLK)]
    for (r, c), w in d.items():
        blocks_by_r[r].append((c, w))
    for lst in blocks_by_r:
        lst.sort()
    return blocks_by_r


def _plan(row_idx, col_idx):
    """ki assignment (+ dup columns) and row pairing."""
    blocks_by_r = _dedupe(row_idx, col_idx)
    ki_of = {}
    for r in range(OUT_BLK):
        for (c, w) in blocks_by_r[r]:
            ki_of[(r, c)] = c % 2

    # global even/odd rebalance via duplicated columns
    total = sum(len(b) for b in blocks_by_r)
    n_even = sum(1 for v in ki_of.values() if v == 0)
    dup_cols = []     # (col, dst_half)
    excess = n_even - (total - n_even)
    if abs(excess) >= 2:
        m = abs(excess) // 2
        src_par = 0 if excess > 0 else 1
        usage = {}
        for (r, c) in ki_of:
            if c % 2 == src_par:
                usage[c] = usage.get(c, 0) + 1
        side = {r: sum(1 for (c, _) in blocks_by_r[r]
                       if ki_of[(r, c)] == src_par)
                for r in range(OUT_BLK)}
        for c in sorted(usage, key=lambda c: -usage[c]):
            if m <= 0:
                break
            dup_cols.append((c, 1 - src_par))
            users = sorted((r for r in range(OUT_BLK) if (r, c) in ki_of),
                           key=lambda r: -side[r])
            for r in users:
                if m <= 0:
                    break
                if ki_of[(r, c)] == src_par:
                    ki_of[(r, c)] = 1 - src_par
                    side[r] -= 1
                    m -= 1

    # pair rows, balancing per-pair ki-0 load
    e_r = [sum(1 for (c, _) in blocks_by_r[r] if ki_of[(r, c)] == 0)
           for r in range(OUT_BLK)]
    order = sorted(range(OUT_BLK), key=lambda r: e_r[r])
    pairs = []
    for p in range(OUT_BLK // 2):
        a, b = order[p], order[OUT_BLK - 1 - p]
        pairs.append((a, b) if p % 2 == 0 else (b, a))
    return blocks_by_r, ki_of, dup_cols, pairs


def _x_resource(b):
    ki, mi, c, w = b
    return ("tile", c // 2) if (c % 2) == ki else ("dup", c)


def _schedule(blocks_by_r, ki_of, dup_cols, pairs, ntok):
    """Greedy discrete-event scheduler -> ordered op list, weight slots,
    x DMA order, and estimated makespan."""
    n_th = ntok // 512

    # units: (pair, th), in pair order
    units = []
    ublocks = []
    for p, (r1, r2) in enumerate(pairs):
        blocks = []
        for mi, r in enumerate((r1, r2)):
            for (c, w) in blocks_by_r[r]:
                blocks.append((ki_of[(r, c)], mi, c, w))
        for th in range(n_th):
            units.append((p, th))
            ublocks.append(blocks)

    # x DMA order: first-use order over units; dups right after source tile
    x_order = []
    seen = set()
    for ub in ublocks:
        for b in ub:
            res = _x_resource(b)
            if res not in seen:
                seen.add(res)
                x_order.append(res)
    for t in range(IN_BLK // 2):
        if ("tile", t) not in seen:
            x_order.append(("tile", t))
            seen.add(("tile", t))

    x_arr = {}
    acc = T_BOOT
    for res in x_order:
        nbytes = (128 if res[0] == "tile" else 64) * ntok * 2
        acc += nbytes / XBW
        x_arr[res] = acc

    wslot = {}
    ops = None
    for pass_i in range(2):
        ops = []
        emit_order = []
        qfree = {(ki, mi): 0.0 for ki in (0, 1) for mi in (0, 1)}
        open_units = []
        unit_gate = {}
        next_unit = 0
        psum_free_t = [0.0] * 4
        ring_pos = 0
        remaining = []
        for ub in ublocks:
            rem = {}
            for b in ub:
                rem.setdefault((b[0], b[1]), []).append(b)
            remaining.append(rem)

        def w_arr(b):
            if pass_i == 0 or (b[3] not in wslot):
                return 0.0
            g = wslot[b[3]][1]
            return T_BOOT + (g + 1) * (128 * WSLOTS * BLK * 2) / WBW

        while next_unit < len(units) or open_units:
            while len(open_units) < 4 and next_unit < len(units):
                gate = psum_free_t[ring_pos % 4]
                ui = next_unit
                open_units.append(ui)
                unit_gate[ui] = gate
                ops.append(("open", ui))
                next_unit += 1
                ring_pos += 1
            best = None
            for q in sorted(qfree, key=lambda q: qfree[q]):
                for ui in open_units:
                    for b in remaining[ui].get(q, []):
                        t0 = max(qfree[q], unit_gate[ui],
                                 x_arr[_x_resource(b)], w_arr(b))
                        left = sum(len(v) for v in remaining[ui].values())
                        key = (t0, left, ui)
                        if best is None or key < best[0]:
                            best = (key, q, ui, b)
                if best is not None and best[0][0] <= qfree[q]:
                    break
            if best is None:
                break
            key, q, ui, b = best
            remaining[ui][q].remove(b)
            ops.append(("mm", ui, b))
            emit_order.append(b)
            qfree[q] = key[0] + MM_NS
            if all(len(v) == 0 for v in remaining[ui].values()):
                ops.append(("close", ui))
                open_units.remove(ui)
                slot = min(range(4), key=lambda s: psum_free_t[s])
                psum_free_t[slot] = key[0] + MM_NS + EVICT_NS
        if pass_i == 0:
            cnt = [0, 0]
            for b in emit_order:
                if b[3] not in wslot:
                    ki = b[0]
                    idx = cnt[ki]
                    cnt[ki] += 1
                    wslot[b[3]] = (ki, idx // WSLOTS, idx % WSLOTS)
    makespan = max(qfree.values())
    n_groups = 0
    for (ki, g, j) in wslot.values():
        n_groups = max(n_groups, g + 1)
    return units, ublocks, ops, wslot, n_groups, x_order, makespan


# ------------------------------------------------------------------- build

def _build(blocks_by_r, ki_of, dup_cols, pairs, units, ops, wslot,
           n_groups, x_order, ntok):
    n_th = ntok // 512
    sdt = mybir.dt.bfloat16
    f32 = mybir.dt.float32
    f16 = mybir.dt.float16
    n_pairs = len(pairs)

    nc = bacc.Bacc("TRN2", target_bir_lowering=False, debug=False)
    xt_d = nc.dram_tensor("xt", [D_IN, ntok], sdt, kind="ExternalInput").ap()
    wg_d = nc.dram_tensor("wg", [max(n_groups, 1), 128, WSLOTS * BLK], sdt,
                          kind="ExternalInput").ap()
    bias_d = nc.dram_tensor("bias_pk", [128, n_pairs], f32,
                            kind="ExternalInput").ap()
    yt_d = nc.dram_tensor("yt", [D_OUT, ntok], f16,
                          kind="ExternalOutput").ap()

    with tile.TileContext(nc) as tc:
        with ExitStack() as ctx:
            xpool = ctx.enter_context(tc.tile_pool(name="xp", bufs=1))
            wpool = ctx.enter_context(tc.tile_pool(name="wp", bufs=1))
            pspool = ctx.enter_context(
                tc.tile_pool(name="ps", bufs=8, space="PSUM"))
            opool = ctx.enter_context(tc.tile_pool(name="op", bufs=10))
            bpool = ctx.enter_context(tc.tile_pool(name="bp", bufs=1))

            bias_sb = bpool.tile([128, n_pairs], f32, tag="bias",
                                 name="bias_sb")
            nc.sync.dma_start(bias_sb[:], bias_d[:])

            # x tiles in first-use order, issues round-robined over
            # sync/scalar (vector cannot issue DMAs)
            xengines = [nc.sync, nc.scalar]
            xtiles = {}
            for i, res in enumerate(x_order):
                eng = xengines[i % 2]
                if res[0] == "tile":
                    t = res[1]
                    xt = xpool.tile([128, ntok], sdt, tag=f"x{t}",
                                    name=f"x{t}")
                    eng.dma_start(xt[:], xt_d[128 * t:128 * (t + 1), :])
                else:
                    c = res[1]
                    half = dict(dup_cols)[c]
                    xt = xpool.tile([128, ntok], sdt, tag=f"xd{c}",
                                    name=f"xd{c}")
                    eng.dma_start(xt[64 * half:64 * half + 64, :],
                                  xt_d[64 * c:64 * (c + 1), :])
                xtiles[res] = xt

            wg_tiles = []
            for g in range(n_groups):
                wt = wpool.tile([128, WSLOTS * BLK], sdt, tag=f"w{g}",
                                name=f"w{g}")
                nc.gpsimd.dma_start(wt[:], wg_d[g, :, :])
                wg_tiles.append(wt)

            def x_ap(b, th):
                ki = b[0]
                t = xtiles[_x_resource(b)]
                return t[ki * 64:(ki + 1) * 64, th * 512:(th + 1) * 512]

            ucount = []
            for p, (r1, r2) in enumerate(pairs):
                cnt = {(ki, mi): 0 for ki in (0, 1) for mi in (0, 1)}
                for mi, r in enumerate((r1, r2)):
                    for (c, w) in blocks_by_r[r]:
                        cnt[(ki_of[(r, c)], mi)] += 1
                ucount.append(cnt)

            psum = {}
            osb_of = {}
            th_done = {}
            done_cnt = {}

            def evict(ui):
                p, th = units[ui]
                if p not in osb_of:
                    osb_of[p] = opool.tile([128, ntok], f16, tag="o",
                                           name=f"o{p}")
                osb = osb_of[p]
                osl = osb[:, th * 512:(th + 1) * 512]
                pt = psum.pop(ui)
                cnt = ucount[p]
                if all(cnt[k] > 0 for k in cnt):
                    nc.scalar.activation(
                        osl, pt[0][:],
                        mybir.ActivationFunctionType.Identity,
                        bias=bias_sb[:, p:p + 1], scale=1.0)
                    nc.vector.tensor_add(osl, osl, pt[1][:])
                else:
                    for mi in (0, 1):
                        osl_h = osl[mi * 64:(mi + 1) * 64, :]
                        bh = bias_sb[mi * 64:(mi + 1) * 64, p:p + 1]
                        srcs = [pt[ki][mi * 64:(mi + 1) * 64, :]
                                for ki in (0, 1) if cnt[(ki, mi)] > 0]
                        if not srcs:
                            nc.vector.memset(osl_h, 0.0)
                            nc.vector.tensor_scalar_add(osl_h, osl_h, bh)
                        else:
                            nc.scalar.activation(
                                osl_h, srcs[0],
                                mybir.ActivationFunctionType.Identity,
                                bias=bh, scale=1.0)
                            if len(srcs) > 1:
                                nc.vector.tensor_add(osl_h, osl_h, srcs[1])
                th_done.setdefault(p, set()).add(th)
                if len(th_done[p]) == n_th:
                    nc.gpsimd.dma_start(yt_d[128 * p:128 * (p + 1), :],
                                        osb_of[p][:])

            for op in ops:
                if op[0] == "open":
                    ui = op[1]
                    psum[ui] = [pspool.tile([128, 512], f32, tag="ps",
                                            name=f"ps{ui}_{k}")
                                for k in range(2)]
                    done_cnt[ui] = {(ki, mi): 0 for ki in (0, 1)
                                    for mi in (0, 1)}
                elif op[0] == "mm":
                    _, ui, b = op
                    ki, mi, c, w = b
                    p, th = units[ui]
                    kis, g, j = wslot[w]
                    lhsT = wg_tiles[g][kis * 64:(kis + 1) * 64,
                                       j * BLK:(j + 1) * BLK]
                    done_cnt[ui][(ki, mi)] += 1
                    first = done_cnt[ui][(ki, mi)] == 1
                    last = done_cnt[ui][(ki, mi)] == ucount[p][(ki, mi)]
                    nc.tensor.matmul(
                        psum[ui][ki][mi * 64:(mi + 1) * 64, :],
                        lhsT, x_ap(b, th),
                        start=first, stop=last,
                        tile_position=(ki * 64, mi * 64),
                        skip_group_check=True,
                    )
                elif op[0] == "close":
                    evict(op[1])

            for p in range(n_pairs):
                if p not in th_done:
                    osb = opool.tile([128, ntok], f16, tag="o",
                                     name=f"oz{p}")
                    for th in range(n_th):
                        osl = osb[:, th * 512:(th + 1) * 512]
                        nc.vector.memset(osl, 0.0)
                        nc.vector.tensor_scalar_add(
                            osl, osl, bias_sb[:, p:p + 1])
                    nc.gpsimd.dma_start(yt_d[128 * p:128 * (p + 1), :],
                                        osb[:])
    nc.compile()
    return nc


# ---------------------------------------------------------------- pack/run

def kernel(x, weight, bias, row_idx, col_idx):
    x = np.asarray(x, dtype=np.float32)
    weight = np.asarray(weight, dtype=np.float32)
    bias = np.asarray(bias, dtype=np.float32)
    row_idx = np.asarray(row_idx)
    col_idx = np.asarray(col_idx)
    ntok_total = x.shape[0]
    assert ntok_total % N_CORES == 0
    ntok = ntok_total // N_CORES
    assert ntok % 512 == 0

    blocks_by_r, ki_of, dup_cols, pairs = _plan(row_idx, col_idx)
    units, ublocks, ops, wslot, n_groups, x_order, makespan = _schedule(
        blocks_by_r, ki_of, dup_cols, pairs, ntok)
    nc = _build(blocks_by_r, ki_of, dup_cols, pairs, units, ops, wslot,
                n_groups, x_order, ntok)

    # pack weights: slot (ki, g, j) at [g, ki*64:(ki+1)*64, j*64:(j+1)*64]
    wg = np.zeros((max(n_groups, 1), 128, WSLOTS * BLK), dtype=BF16)
    wT = np.ascontiguousarray(
        np.transpose(weight, (0, 2, 1))).astype(BF16)
    for w, (ki, g, j) in wslot.items():
        wg[g, ki * 64:(ki + 1) * 64, j * BLK:(j + 1) * BLK] = wT[w]

    bias_pk = np.zeros((128, len(pairs)), dtype=np.float32)
    for p, (r1, r2) in enumerate(pairs):
        bias_pk[:64, p] = bias[r1 * BLK:(r1 + 1) * BLK]
        bias_pk[64:, p] = bias[r2 * BLK:(r2 + 1) * BLK]

    in_maps = []
    for cid in range(N_CORES):
        xt = np.ascontiguousarray(
            x[cid * ntok:(cid + 1) * ntok].T).astype(BF16)
        in_maps.append({"xt": xt, "wg": wg, "bias_pk": bias_pk})

    res = run_bass_kernel_spmd(nc, in_maps, core_ids=list(range(N_CORES)))

    # un-permute: DRAM rows [128p, 128p+128) hold pair p's two row-blocks
    perm = np.empty(D_OUT, dtype=np.int64)
    for p, (r1, r2) in enumerate(pairs):
        perm[r1 * BLK:(r1 + 1) * BLK] = np.arange(128 * p, 128 * p + 64)
        perm[r2 * BLK:(r2 + 1) * BLK] = np.arange(128 * p + 64,
                                                  128 * p + 128)
    y = np.empty((ntok_total, D_OUT), dtype=np.float32)
    for cid in range(N_CORES):
        yt = res.results[cid]["yt"]
        y[cid * ntok:(cid + 1) * ntok] = yt[perm, :].T.astype(np.float32)
    return y
